# revision 28
# baseline (speedup 1.0000x reference)
"""Grouped self-attention (GQA) Trainium2 kernel, v5.

Problem: B=2, T=2048, D=2048, 16 Q heads / 4 KV heads, head_dim=128,
full RoPE (base 1e6), causal softmax, output projection.

Sharding: 8 cores = 2 batches x 4 KV groups. Core c handles batch c//4,
kv-group c%4 (4 Q heads + 1 KV head). q/k/v projections column-sharded,
o_proj row-sharded; per-core partial outputs are summed on host.

v5 (vs v4):
- projection PSUM tiles are evicted by a single ACT copy to fp16 SBUF
  (bank frees in ~0.7us; v4 held banks through ~2-6us of queued DVE
  rope reads, stalling the next pass's matmuls in low p-state).
- RoPE applied on the fp16 SBUF copy with fp16 cos/sin tables: the
  cos-multiply runs on gpsimd, the two rotate-half sin-multiplies and
  the final add on DVE (all-SBUF 16-bit ops are ~2x faster than the
  old f32 PSUM reads, and the work is split across two engines).
- softmax strip-accumulation on DVE (gpsimd tensor ops measured ~1us
  per [128,512] — too slow for the serial R chain); the denominator
  ones-matmul is emitted after the interleaved q-projection pass so
  the R chain drains while the PE is busy.
- o-proj: the first two (tb,nci) units accumulate heads 0-2 first,
  deferring their h3 matmuls until h3's normalization chain lands.
- v-projection PSUM tiles declared before k's so the phase-1b q-pair
  pool lands on banks freed by fast ACT copies.
"""

import os
import sys

import numpy as np

for _p in ("/opt/trn_rl_repo",):
    if _p not in sys.path and os.path.isdir(_p):
        sys.path.insert(0, _p)

import concourse.bass as bass  # noqa: E402
import concourse.mybir as mybir  # noqa: E402
import concourse.tile as tile  # noqa: E402
from concourse import bacc  # noqa: E402
from concourse.bass_utils import run_bass_kernel_spmd  # noqa: E402
from concourse.masks import make_identity  # noqa: E402

B, T, D = 2, 2048, 2048
NH, NKV, HD = 16, 4, 128
G = NKV              # kv groups == cores per batch
AQ = (NH // NKV) * HD  # attention cols per core (4 heads x 128)
KC = D // 128        # 16 contraction chunks for projections
ROPE_BASE = 1000000.0
INV_SQRT_D = 1.0 / float(np.sqrt(HD))

F32 = mybir.dt.float32
FP16 = mybir.dt.float16

PT_MAX = 512 * 13 + 768  # widest per-(g,h) P^T row (g=3): 7424


def _strips(g):
    """Score strips for tq group g (cols [512g, 512g+512)).

    Returns [(j, off, w)]: kv block j contributes group columns
    [512-w, 512); off is the strip's offset in the packed P^T buffer.
    """
    out = []
    off = 0
    for j in range(4 * g + 4):
        w = 512 - max(0, 128 * j - 512 * g)
        out.append((j, off, w))
        off += w
    return out


_CACHE = {}


def _build_nc():
    nc = bacc.Bacc(None, target_bir_lowering=False, debug=False)

    # host-packed inputs (see kernel() for layouts)
    xp_d = nc.dram_tensor("xp", [128, KC, T], FP16, kind="ExternalInput")
    wp_d = nc.dram_tensor("wp", [128, KC, 768], FP16, kind="ExternalInput")
    wo_d = nc.dram_tensor("wo", [128, 4, D], FP16, kind="ExternalInput")
    cos_d = nc.dram_tensor("cosT", [HD, T], FP16, kind="ExternalInput")
    sin_d = nc.dram_tensor("sinT", [HD, T], FP16, kind="ExternalInput")
    mask_d = nc.dram_tensor("mask", [128, 128], F32, kind="ExternalInput")
    y_d = nc.dram_tensor("y", [T, D], FP16, kind="ExternalOutput")

    mult = mybir.AluOpType.mult
    add = mybir.AluOpType.add
    Exp = mybir.ActivationFunctionType.Exp

    with tile.TileContext(nc) as tc:
        with (
            tc.tile_pool(name="const", bufs=1) as cpool,
            tc.tile_pool(name="qkv", bufs=1) as qkv_pool,
            tc.tile_pool(name="xw", bufs=1) as xw_pool,
            tc.tile_pool(name="ptmp", bufs=3) as tmp_pool,
            # single PSUM pool for the whole kernel: 8 one-bank slots
            # s0..s7 managed by tag. Mid-kernel pool closes insert
            # conservative all-bank barriers (cost ~8us each); explicit
            # slot reuse keeps dependencies per-bank instead.
            tc.tile_pool(name="pp", bufs=1, space="PSUM") as pp,
        ):
            cos_sb = cpool.tile([HD, T], FP16, tag="cos")
            sin_sb = cpool.tile([HD, T], FP16, tag="sin")
            mask_sb = cpool.tile([128, 128], F32, tag="mask")
            id_fp = cpool.tile([128, 128], FP16, tag="idf")
            ones_sb = cpool.tile([128, 1], FP16, tag="ones")
            wo_sb = cpool.tile([128, 4, D], FP16, tag="wo")

            xt = xw_pool.tile([128, KC, T], FP16, tag="xt")
            w_all = xw_pool.tile([128, KC, 768], FP16, tag="w")
            # x/w stream per chunk, interleaved, on the sync queue;
            # everything else on the gpsimd queue.
            nc.gpsimd.dma_start(mask_sb[:], mask_d[:])
            for e in range(KC):
                nc.sync.dma_start(w_all[:, e, :], wp_d[:, e, :])
                nc.sync.dma_start(xt[:, e, :], xp_d[:, e, :])
            nc.gpsimd.dma_start(cos_sb[:], cos_d[:])
            nc.gpsimd.dma_start(sin_sb[:], sin_d[:])
            nc.gpsimd.dma_start(wo_sb[:], wo_d[:])
            make_identity(nc, id_fp[:])
            nc.gpsimd.memset(ones_sb[:], 1.0)

            qT = qkv_pool.tile([128, 4, T], FP16, tag="qT")   # [d, h, t]
            kT = qkv_pool.tile([128, T], FP16, tag="kT")      # [d, t]
            v_sb = qkv_pool.tile([128, T], FP16, tag="v")     # [tk%128, blk*128+d]
            vT_sb = qkv_pool.tile([128, T], FP16, tag="vT")   # [d, t] pre-transpose

            def rope_evict(ps, dst, tsl):
                """Evict the projection PSUM tile via three ACT copies
                (straight + rotate-half, freeing the bank fast), then
                partition-aligned fp16 rope on gpsimd/DVE:
                dst = raw*cos + rot*sin  (sin table carries the
                rotate-half sign)."""
                raw = tmp_pool.tile([128, 512], FP16, tag="qraw")
                rot = tmp_pool.tile([128, 512], FP16, tag="qrot")
                t1 = tmp_pool.tile([128, 512], FP16, tag="ropetmp")
                nc.scalar.copy(raw[:], ps[:])
                nc.scalar.copy(rot[0:64, :], ps[64:128, :])
                nc.scalar.copy(rot[64:128, :], ps[0:64, :])
                # NOTE: keep gpsimd free of tensor ops — the Pool DSP
                # swaps microcode libraries between op families
                # (UNLOAD_LIB/LOAD_LIB, ~6.6us each) and thrashes if it
                # alternates tensor_tensor with partition_broadcast.
                nc.vector.tensor_tensor(t1[:], raw[:], cos_sb[:, tsl], mult)
                nc.vector.tensor_tensor(dst[:], rot[:], sin_sb[:, tsl], mult)
                nc.vector.tensor_tensor(dst[:], dst[:], t1[:], add)

            # ---- phase 1a: k+v projections, e-outer (DMA-streamed) ----
            # slots s0..s3: v accumulators; s4..s7: k accumulators
            psv = [pp.tile([128, 512], F32, tag=f"s{t}",
                           name=f"psv{t}") for t in range(4)]
            psk = [pp.tile([128, 512], F32, tag=f"s{4 + t}",
                           name=f"psk{t}") for t in range(4)]
            for e in range(KC):
                for tci in range(4):
                    nc.tensor.matmul(
                        psv[tci][:], w_all[:, e, 640:768],
                        xt[:, e, tci * 512:(tci + 1) * 512],
                        start=(e == 0), stop=(e == KC - 1))
                for tci in range(4):
                    nc.tensor.matmul(
                        psk[tci][:], w_all[:, e, 512:640],
                        xt[:, e, tci * 512:(tci + 1) * 512],
                        start=(e == 0), stop=(e == KC - 1))
            # vT first: 4 fast ACT copies release the psv slots the
            # phase-1b q-pairs reuse; k ropes then run on the (idle)
            # DVE straight from PSUM, freeing psk slots without
            # touching the ACT queue.
            for tci in range(4):
                tsl = slice(tci * 512, (tci + 1) * 512)
                nc.scalar.copy(vT_sb[:, tsl], psv[tci][:])
            for tci in range(4):
                tsl = slice(tci * 512, (tci + 1) * 512)
                ps = psk[tci]
                dst = kT[:, tsl]
                t1 = tmp_pool.tile([128, 512], FP16, tag="ropetmp")
                nc.vector.tensor_tensor(
                    t1[:], ps[:], cos_sb[:, tsl], mult)
                nc.vector.tensor_tensor(
                    dst[0:64, :], ps[64:128, :], sin_sb[0:64, tsl], mult)
                nc.vector.tensor_tensor(
                    dst[64:128, :], ps[0:64, :], sin_sb[64:128, tsl], mult)
                nc.vector.tensor_tensor(dst[:], dst[:], t1[:], add)

            # ---- phase 1b: q tci0 head-pairs, v transpose ----
            def q_pass_pair(ha, tci, ta, tb):
                tsl = slice(tci * 512, (tci + 1) * 512)
                pa = pp.tile([128, 512], F32, tag=ta,
                             name=f"q{ha}_{tci}")
                pb = pp.tile([128, 512], F32, tag=tb,
                             name=f"q{ha + 1}_{tci}")
                for e in range(KC):
                    nc.tensor.matmul(
                        pa[:], w_all[:, e, ha * 128:(ha + 1) * 128],
                        xt[:, e, tsl], start=(e == 0), stop=(e == KC - 1))
                    nc.tensor.matmul(
                        pb[:], w_all[:, e, (ha + 1) * 128:(ha + 2) * 128],
                        xt[:, e, tsl], start=(e == 0), stop=(e == KC - 1))
                rope_evict(pa, qT[:, ha, tsl], tsl)
                rope_evict(pb, qT[:, ha + 1, tsl], tsl)

            def rope_evict_dve(ps, dst, tsl):
                """All-DVE rope straight from PSUM (partition-shifted
                reads are legal with a PSUM operand). Used where the
                ACT queue is the binding resource (attention-era q
                passes: ACT copies there delay the next head's exps)."""
                t1 = tmp_pool.tile([128, 512], FP16, tag="ropetmp")
                nc.vector.tensor_tensor(t1[:], ps[:], cos_sb[:, tsl], mult)
                nc.vector.tensor_tensor(
                    dst[0:64, :], ps[64:128, :], sin_sb[0:64, tsl], mult)
                nc.vector.tensor_tensor(
                    dst[64:128, :], ps[0:64, :], sin_sb[64:128, tsl], mult)
                nc.vector.tensor_tensor(dst[:], dst[:], t1[:], add)

            def q_pass(h, tci):
                tsl = slice(tci * 512, (tci + 1) * 512)
                ps = pp.tile([128, 512], F32, tag="s3",
                             name=f"q{h}_{tci}")
                for e in range(KC):
                    nc.tensor.matmul(
                        ps[:], w_all[:, e, h * 128:(h + 1) * 128],
                        xt[:, e, tsl], start=(e == 0), stop=(e == KC - 1))
                rope_evict_dve(ps, qT[:, h, tsl], tsl)

            q_pass_pair(0, 0, "s0", "s1")
            q_pass_pair(2, 0, "s2", "s3")
            for tci in range(4):
                tsl = slice(tci * 512, (tci + 1) * 512)
                pst = pp.tile([128, 512], FP16, tag=f"s{4 + tci % 2}",
                              name=f"vtr{tci}")
                for j4 in range(4):
                    nc.tensor.transpose(
                        pst[:, j4 * 128:(j4 + 1) * 128],
                        vT_sb[:, tci * 512 + j4 * 128:
                              tci * 512 + (j4 + 1) * 128],
                        id_fp[:],
                    )
                nc.vector.tensor_copy(v_sb[:, tsl], pst[:])

            # ---- attention, tq-group-major, o-proj per group ----
            # slot plan: ST s4/s5 (alternating per strip), OT s6/s7
            # (per head), SUM s2, interleaved q passes s3, Y s0/s1.
            with (
                tc.tile_pool(name="att", bufs=2) as att_pool,
                tc.tile_pool(name="small", bufs=2) as small_pool,
                tc.tile_pool(name="yev", bufs=4) as yev_pool,
            ):
                cp = 0
                for g in range(4):
                    sl = _strips(g)
                    n = len(sl)
                    OTg = att_pool.tile([128, 4, 512], FP16, tag="OTg")
                    for h in range(4):
                        PT = att_pool.tile([128, PT_MAX], FP16, tag="PT")
                        R0 = att_pool.tile([128, 512], FP16, tag="R0")
                        pso = pp.tile([128, 512], F32,
                                      tag=f"s{6 + (4 * g + h) % 2}",
                                      name=f"ot_{g}_{h}")

                        def ot_mm(idx):
                            j, off, w = sl[idx]
                            nc.tensor.matmul(
                                pso[:, 512 - w:512],
                                v_sb[:, j * 128:(j + 1) * 128],
                                PT[:, off:off + w],
                                start=(idx == 0),
                                stop=(idx == n - 1),
                                skip_group_check=True,
                            )

                        for idx, (j, off, w) in enumerate(sl):
                            ps = pp.tile([128, 512], F32,
                                         tag=f"s{4 + idx % 2}",
                                         name=f"st_{g}_{h}_{idx}")
                            tq0 = max(512 * g, 128 * j)
                            nc.tensor.matmul(
                                ps[:, :w],
                                kT[:, j * 128:(j + 1) * 128],
                                qT[:, h, tq0:512 * g + 512],
                                start=True, stop=True,
                                skip_group_check=True)
                            if j >= 4 * g:
                                nc.vector.tensor_tensor(
                                    ps[:, :128], ps[:, :128],
                                    mask_sb[:], add)
                            nc.scalar.activation(
                                PT[:, off:off + w], ps[:, :w], Exp,
                                scale=INV_SQRT_D)
                            # strip accumulation for softmax sums (DVE)
                            if idx == 0:
                                nc.vector.tensor_copy(R0[:], PT[:, 0:512])
                            else:
                                nc.vector.tensor_tensor(
                                    R0[:, 512 - w:], R0[:, 512 - w:],
                                    PT[:, off:off + w], add)
                            if idx >= 2:
                                ot_mm(idx - 2)
                        if n >= 2:
                            ot_mm(n - 2)
                        ot_mm(n - 1)
                        # denominator chain first (s_sb on the idle ACT,
                        # recip/evict at the FRONT of the DVE queue),
                        # then the q-projection pass: its 16 matmuls keep
                        # the PE busy while the chain completes, and its
                        # rope queues BEHIND the chain on DVE instead of
                        # interleaving into it.
                        ps1 = pp.tile([1, 512], F32, tag="s2",
                                      name=f"sum_{g}_{h}")
                        nc.tensor.matmul(
                            ps1[:], ones_sb[:], R0[:], start=True, stop=True)
                        s_sb = small_pool.tile([1, 512], F32, tag="s")
                        nc.scalar.copy(s_sb[:], ps1[:])
                        bc = small_pool.tile([128, 512], F32, tag="bc")
                        nc.gpsimd.partition_broadcast(bc[:], s_sb[:])
                        rcp = small_pool.tile([128, 512], F32, tag="rcp")
                        nc.vector.reciprocal_approx_fast(rcp[:], bc[:])
                        nc.vector.tensor_tensor(
                            OTg[:, h, :], pso[:], rcp[:], mult)
                        if g < 3:
                            q_pass(h, g + 1)
                    # o-proj for this tq group; defer h3 of the first two
                    # units so h3's normalization chain can land
                    units = [(tb, nci) for tb in range(4) for nci in range(4)]
                    psys = {}

                    def oproj_mms(u, hs):
                        tb, nci = u
                        if u not in psys:
                            psys[u] = pp.tile(
                                [128, 512], F32,
                                tag=f"s{units.index(u) % 2}",
                                name=f"y_{g}_{tb}_{nci}")
                        for h in hs:
                            nc.tensor.matmul(
                                psys[u][:],
                                OTg[:, h, tb * 128:(tb + 1) * 128],
                                wo_sb[:, h, nci * 512:(nci + 1) * 512],
                                start=(h == 0), stop=(h == 3),
                                skip_group_check=True)

                    def oproj_evict(u):
                        tb, nci = u
                        ysb = yev_pool.tile([128, 512], FP16, tag="ysb")
                        nonlocal cp
                        if cp % 4 == 0:
                            nc.scalar.copy(ysb[:], psys[u][:])
                        else:
                            nc.vector.tensor_copy(ysb[:], psys[u][:])
                        cp += 1
                        dma_eng = nc.sync if cp % 2 == 0 else nc.gpsimd
                        dma_eng.dma_start(
                            y_d[512 * g + tb * 128:512 * g + tb * 128 + 128,
                                nci * 512:(nci + 1) * 512],
                            ysb[:])

                    oproj_mms(units[0], [0, 1, 2])
                    oproj_mms(units[1], [0, 1, 2])
                    oproj_mms(units[0], [3])
                    oproj_evict(units[0])
                    oproj_mms(units[1], [3])
                    oproj_evict(units[1])
                    for u in units[2:]:
                        oproj_mms(u, [0, 1, 2, 3])
                        oproj_evict(u)

    nc.compile()
    return nc


def _rope_tables():
    pos = np.arange(T, dtype=np.float32)
    inv_freq = (1.0 / (ROPE_BASE ** (np.arange(0, HD, 2, dtype=np.float32) / HD))).astype(np.float32)
    ang = pos[:, None] * inv_freq[None, :]            # [T, 64]
    cos = np.cos(ang).astype(np.float32)
    sin = np.sin(ang).astype(np.float32)
    cosT = np.ascontiguousarray(np.concatenate([cos, cos], 1).T)   # [128, T]
    sinT = np.ascontiguousarray(np.concatenate([-sin, sin], 1).T)  # rotate_half sign
    return cosT.astype(np.float16), sinT.astype(np.float16)


def kernel(x, Wq, bq, Wk, bk, Wv, bv, Wo, bo, **_ignored):
    x = np.asarray(x, dtype=np.float32)
    Wq = np.asarray(Wq, dtype=np.float32)
    Wk = np.asarray(Wk, dtype=np.float32)
    Wv = np.asarray(Wv, dtype=np.float32)
    Wo = np.asarray(Wo, dtype=np.float32)
    bo = np.asarray(bo, dtype=np.float32)

    if "nc" not in _CACHE:
        _CACHE["nc"] = _build_nc()
    nc = _CACHE["nc"]

    cosT, sinT = _rope_tables()
    # S^T layout: mask[tk, tq] allows tk <= tq within the diagonal block
    triu = np.triu(np.ones((128, 128), dtype=bool))
    mask = np.where(triu, 0.0, -1e9).astype(np.float32)

    in_maps = []
    for c in range(8):
        b, g = c // G, c % G
        xT = x[b].T.astype(np.float16)                  # [D, T]
        xp = np.ascontiguousarray(
            xT.reshape(KC, 128, T).transpose(1, 0, 2))
        wq = Wq[:, g * AQ:(g + 1) * AQ].astype(np.float16)
        wk = Wk[:, g * HD:(g + 1) * HD].astype(np.float16)
        wv = Wv[:, g * HD:(g + 1) * HD].astype(np.float16)
        wcat = np.concatenate([wq, wk, wv], axis=1)     # [D, 768]
        wp = np.ascontiguousarray(
            wcat.reshape(KC, 128, 768).transpose(1, 0, 2))
        wo = np.ascontiguousarray(
            Wo[g * AQ:(g + 1) * AQ, :].astype(np.float16)
            .reshape(4, 128, D).transpose(1, 0, 2))
        in_maps.append({
            "xp": xp,
            "wp": wp,
            "wo": wo,
            "cosT": cosT,
            "sinT": sinT,
            "mask": mask,
        })

    res = run_bass_kernel_spmd(
        nc, in_maps, list(range(8)),
        trace=bool(os.environ.get("KERNEL_TRACE")),
        tmpdir=os.environ.get("KERNEL_TRACE_DIR") or None,
    )
    _CACHE["last_results"] = res

    out = np.zeros((B, T, D), dtype=np.float32)
    for b in range(B):
        acc = np.zeros((T, D), dtype=np.float32)
        for g in range(G):
            acc += res.results[b * G + g]["y"].astype(np.float32)
        out[b] = acc + bo[None, :]
    return out


# revision 32
# speedup vs baseline: 1.0015x; 1.0015x over previous
"""Grouped self-attention (GQA) Trainium2 kernel, v5.

Problem: B=2, T=2048, D=2048, 16 Q heads / 4 KV heads, head_dim=128,
full RoPE (base 1e6), causal softmax, output projection.

Sharding: 8 cores = 2 batches x 4 KV groups. Core c handles batch c//4,
kv-group c%4 (4 Q heads + 1 KV head). q/k/v projections column-sharded,
o_proj row-sharded; per-core partial outputs are summed on host.

v5 (vs v4):
- projection PSUM tiles are evicted by a single ACT copy to fp16 SBUF
  (bank frees in ~0.7us; v4 held banks through ~2-6us of queued DVE
  rope reads, stalling the next pass's matmuls in low p-state).
- RoPE applied on the fp16 SBUF copy with fp16 cos/sin tables: the
  cos-multiply runs on gpsimd, the two rotate-half sin-multiplies and
  the final add on DVE (all-SBUF 16-bit ops are ~2x faster than the
  old f32 PSUM reads, and the work is split across two engines).
- softmax strip-accumulation on DVE (gpsimd tensor ops measured ~1us
  per [128,512] — too slow for the serial R chain); the denominator
  ones-matmul is emitted after the interleaved q-projection pass so
  the R chain drains while the PE is busy.
- o-proj: the first two (tb,nci) units accumulate heads 0-2 first,
  deferring their h3 matmuls until h3's normalization chain lands.
- v-projection PSUM tiles declared before k's so the phase-1b q-pair
  pool lands on banks freed by fast ACT copies.
"""

import os
import sys

import numpy as np

for _p in ("/opt/trn_rl_repo",):
    if _p not in sys.path and os.path.isdir(_p):
        sys.path.insert(0, _p)

import concourse.bass as bass  # noqa: E402
import concourse.mybir as mybir  # noqa: E402
import concourse.tile as tile  # noqa: E402
from concourse import bacc  # noqa: E402
from concourse.bass_utils import run_bass_kernel_spmd  # noqa: E402
from concourse.masks import make_identity  # noqa: E402

B, T, D = 2, 2048, 2048
NH, NKV, HD = 16, 4, 128
G = NKV              # kv groups == cores per batch
AQ = (NH // NKV) * HD  # attention cols per core (4 heads x 128)
KC = D // 128        # 16 contraction chunks for projections
ROPE_BASE = 1000000.0
INV_SQRT_D = 1.0 / float(np.sqrt(HD))

F32 = mybir.dt.float32
FP16 = mybir.dt.float16

PT_MAX = 512 * 13 + 768  # widest per-(g,h) P^T row (g=3): 7424


def _strips(g):
    """Score strips for tq group g (cols [512g, 512g+512)).

    Returns [(j, off, w)]: kv block j contributes group columns
    [512-w, 512); off is the strip's offset in the packed P^T buffer.
    """
    out = []
    off = 0
    for j in range(4 * g + 4):
        w = 512 - max(0, 128 * j - 512 * g)
        out.append((j, off, w))
        off += w
    return out


_CACHE = {}


def _build_nc():
    nc = bacc.Bacc(None, target_bir_lowering=False, debug=False)

    # host-packed inputs (see kernel() for layouts)
    xp_d = nc.dram_tensor("xp", [128, KC, T], FP16, kind="ExternalInput")
    wp_d = nc.dram_tensor("wp", [128, KC, 768], FP16, kind="ExternalInput")
    wo_d = nc.dram_tensor("wo", [128, 4, D], FP16, kind="ExternalInput")
    cos_d = nc.dram_tensor("cosT", [HD, T], FP16, kind="ExternalInput")
    sin_d = nc.dram_tensor("sinT", [HD, T], FP16, kind="ExternalInput")
    mask_d = nc.dram_tensor("mask", [128, 128], F32, kind="ExternalInput")
    y_d = nc.dram_tensor("y", [T, D], FP16, kind="ExternalOutput")

    mult = mybir.AluOpType.mult
    add = mybir.AluOpType.add
    Exp = mybir.ActivationFunctionType.Exp

    with tile.TileContext(nc) as tc:
        with (
            tc.tile_pool(name="const", bufs=1) as cpool,
            tc.tile_pool(name="qkv", bufs=1) as qkv_pool,
            tc.tile_pool(name="xw", bufs=1) as xw_pool,
            tc.tile_pool(name="ptmp", bufs=3) as tmp_pool,
            # single PSUM pool for the whole kernel: 8 one-bank slots
            # s0..s7 managed by tag. Mid-kernel pool closes insert
            # conservative all-bank barriers (cost ~8us each); explicit
            # slot reuse keeps dependencies per-bank instead.
            tc.tile_pool(name="pp", bufs=1, space="PSUM") as pp,
        ):
            cos_sb = cpool.tile([HD, T], FP16, tag="cos")
            sin_sb = cpool.tile([HD, T], FP16, tag="sin")
            mask_sb = cpool.tile([128, 128], F32, tag="mask")
            id_fp = cpool.tile([128, 128], FP16, tag="idf")
            ones_sb = cpool.tile([128, 1], FP16, tag="ones")
            wo_sb = cpool.tile([128, 4, D], FP16, tag="wo")

            xt = xw_pool.tile([128, KC, T], FP16, tag="xt")
            w_all = xw_pool.tile([128, KC, 768], FP16, tag="w")
            # x/w stream per chunk, interleaved, on the sync queue;
            # everything else on the gpsimd queue.
            nc.gpsimd.dma_start(mask_sb[:], mask_d[:])
            for e in range(KC):
                nc.sync.dma_start(w_all[:, e, :], wp_d[:, e, :])
                nc.sync.dma_start(xt[:, e, :], xp_d[:, e, :])
            nc.gpsimd.dma_start(cos_sb[:], cos_d[:])
            nc.gpsimd.dma_start(sin_sb[:], sin_d[:])
            nc.gpsimd.dma_start(wo_sb[:], wo_d[:])
            make_identity(nc, id_fp[:])
            nc.gpsimd.memset(ones_sb[:], 1.0)

            qT = qkv_pool.tile([128, 4, T], FP16, tag="qT")   # [d, h, t]
            kT = qkv_pool.tile([128, T], FP16, tag="kT")      # [d, t]
            v_sb = qkv_pool.tile([128, T], FP16, tag="v")     # [tk%128, blk*128+d]
            vT_sb = qkv_pool.tile([128, T], FP16, tag="vT")   # [d, t] pre-transpose

            def rope_evict(ps, dst, tsl):
                """Evict the projection PSUM tile via three ACT copies
                (straight + rotate-half, freeing the bank fast), then
                partition-aligned fp16 rope on gpsimd/DVE:
                dst = raw*cos + rot*sin  (sin table carries the
                rotate-half sign)."""
                raw = tmp_pool.tile([128, 512], FP16, tag="qraw")
                rot = tmp_pool.tile([128, 512], FP16, tag="qrot")
                t1 = tmp_pool.tile([128, 512], FP16, tag="ropetmp")
                nc.scalar.copy(raw[:], ps[:])
                nc.scalar.copy(rot[0:64, :], ps[64:128, :])
                nc.scalar.copy(rot[64:128, :], ps[0:64, :])
                # NOTE: keep gpsimd free of tensor ops — the Pool DSP
                # swaps microcode libraries between op families
                # (UNLOAD_LIB/LOAD_LIB, ~6.6us each) and thrashes if it
                # alternates tensor_tensor with partition_broadcast.
                nc.vector.tensor_tensor(t1[:], raw[:], cos_sb[:, tsl], mult)
                nc.vector.tensor_tensor(dst[:], rot[:], sin_sb[:, tsl], mult)
                nc.vector.tensor_tensor(dst[:], dst[:], t1[:], add)

            # ---- phase 1a: k+v projections, e-outer (DMA-streamed) ----
            # slots s0..s3: v accumulators; s4..s7: k accumulators
            psv = [pp.tile([128, 512], F32, tag=f"s{t}",
                           name=f"psv{t}") for t in range(4)]
            psk = [pp.tile([128, 512], F32, tag=f"s{4 + t}",
                           name=f"psk{t}") for t in range(4)]
            for e in range(KC):
                for tci in range(4):
                    nc.tensor.matmul(
                        psv[tci][:], w_all[:, e, 640:768],
                        xt[:, e, tci * 512:(tci + 1) * 512],
                        start=(e == 0), stop=(e == KC - 1))
                for tci in range(4):
                    nc.tensor.matmul(
                        psk[tci][:], w_all[:, e, 512:640],
                        xt[:, e, tci * 512:(tci + 1) * 512],
                        start=(e == 0), stop=(e == KC - 1))
            # vT first: 4 fast ACT copies release the psv slots the
            # phase-1b q-pairs reuse; k ropes then run on the (idle)
            # DVE straight from PSUM, freeing psk slots without
            # touching the ACT queue.
            for tci in range(4):
                tsl = slice(tci * 512, (tci + 1) * 512)
                nc.scalar.copy(vT_sb[:, tsl], psv[tci][:])
            for tci in range(4):
                tsl = slice(tci * 512, (tci + 1) * 512)
                ps = psk[tci]
                dst = kT[:, tsl]
                t1 = tmp_pool.tile([128, 512], FP16, tag="ropetmp")
                nc.vector.tensor_tensor(
                    t1[:], ps[:], cos_sb[:, tsl], mult)
                nc.vector.tensor_tensor(
                    dst[0:64, :], ps[64:128, :], sin_sb[0:64, tsl], mult)
                nc.vector.tensor_tensor(
                    dst[64:128, :], ps[0:64, :], sin_sb[64:128, tsl], mult)
                nc.vector.tensor_tensor(dst[:], dst[:], t1[:], add)

            # ---- phase 1b: q tci0 head-pairs, v transpose ----
            def q_pass_pair(ha, tci, ta, tb):
                tsl = slice(tci * 512, (tci + 1) * 512)
                pa = pp.tile([128, 512], F32, tag=ta,
                             name=f"q{ha}_{tci}")
                pb = pp.tile([128, 512], F32, tag=tb,
                             name=f"q{ha + 1}_{tci}")
                for e in range(KC):
                    nc.tensor.matmul(
                        pa[:], w_all[:, e, ha * 128:(ha + 1) * 128],
                        xt[:, e, tsl], start=(e == 0), stop=(e == KC - 1))
                    nc.tensor.matmul(
                        pb[:], w_all[:, e, (ha + 1) * 128:(ha + 2) * 128],
                        xt[:, e, tsl], start=(e == 0), stop=(e == KC - 1))
                rope_evict(pa, qT[:, ha, tsl], tsl)
                rope_evict(pb, qT[:, ha + 1, tsl], tsl)

            def rope_evict_dve(ps, dst, tsl):
                """All-DVE rope straight from PSUM (partition-shifted
                reads are legal with a PSUM operand). Used where the
                ACT queue is the binding resource (attention-era q
                passes: ACT copies there delay the next head's exps)."""
                t1 = tmp_pool.tile([128, 512], FP16, tag="ropetmp")
                nc.vector.tensor_tensor(t1[:], ps[:], cos_sb[:, tsl], mult)
                nc.vector.tensor_tensor(
                    dst[0:64, :], ps[64:128, :], sin_sb[0:64, tsl], mult)
                nc.vector.tensor_tensor(
                    dst[64:128, :], ps[0:64, :], sin_sb[64:128, tsl], mult)
                nc.vector.tensor_tensor(dst[:], dst[:], t1[:], add)

            def q_pass_mms(h, tci):
                """Emit only the projection matmuls; the rope eviction
                is deferred past the normalization chain so its DVE ops
                queue BEHIND recip/evict instead of inside the chain."""
                tsl = slice(tci * 512, (tci + 1) * 512)
                ps = pp.tile([128, 512], F32, tag="s3",
                             name=f"q{h}_{tci}")
                for e in range(KC):
                    nc.tensor.matmul(
                        ps[:], w_all[:, e, h * 128:(h + 1) * 128],
                        xt[:, e, tsl], start=(e == 0), stop=(e == KC - 1))
                return ps, tsl

            q_pass_pair(0, 0, "s0", "s1")
            q_pass_pair(2, 0, "s2", "s3")
            for tci in range(4):
                tsl = slice(tci * 512, (tci + 1) * 512)
                pst = pp.tile([128, 512], FP16, tag=f"s{4 + tci % 2}",
                              name=f"vtr{tci}")
                for j4 in range(4):
                    nc.tensor.transpose(
                        pst[:, j4 * 128:(j4 + 1) * 128],
                        vT_sb[:, tci * 512 + j4 * 128:
                              tci * 512 + (j4 + 1) * 128],
                        id_fp[:],
                    )
                nc.vector.tensor_copy(v_sb[:, tsl], pst[:])

            # ---- attention, tq-group-major, o-proj per group ----
            # slot plan: ST s4/s5 (alternating per strip), OT s6/s7
            # (per head), SUM s2, interleaved q passes s3, Y s0/s1.
            with (
                tc.tile_pool(name="att", bufs=2) as att_pool,
                tc.tile_pool(name="small", bufs=2) as small_pool,
                tc.tile_pool(name="yev", bufs=4) as yev_pool,
            ):
                cp = 0
                for g in range(4):
                    sl = _strips(g)
                    n = len(sl)
                    OTg = att_pool.tile([128, 4, 512], FP16, tag="OTg")
                    for h in range(4):
                        PT = att_pool.tile([128, PT_MAX], FP16, tag="PT")
                        R0 = att_pool.tile([128, 512], FP16, tag="R0")
                        pso = pp.tile([128, 512], F32,
                                      tag=f"s{6 + (4 * g + h) % 2}",
                                      name=f"ot_{g}_{h}")

                        def ot_mm(idx):
                            j, off, w = sl[idx]
                            nc.tensor.matmul(
                                pso[:, 512 - w:512],
                                v_sb[:, j * 128:(j + 1) * 128],
                                PT[:, off:off + w],
                                start=(idx == 0),
                                stop=(idx == n - 1),
                                skip_group_check=True,
                            )

                        for idx, (j, off, w) in enumerate(sl):
                            ps = pp.tile([128, 512], F32,
                                         tag=f"s{4 + idx % 2}",
                                         name=f"st_{g}_{h}_{idx}")
                            tq0 = max(512 * g, 128 * j)
                            nc.tensor.matmul(
                                ps[:, :w],
                                kT[:, j * 128:(j + 1) * 128],
                                qT[:, h, tq0:512 * g + 512],
                                start=True, stop=True,
                                skip_group_check=True)
                            if j >= 4 * g:
                                nc.vector.tensor_tensor(
                                    ps[:, :128], ps[:, :128],
                                    mask_sb[:], add)
                            nc.scalar.activation(
                                PT[:, off:off + w], ps[:, :w], Exp,
                                scale=INV_SQRT_D)
                            # strip accumulation for softmax sums (DVE)
                            if idx == 0:
                                nc.vector.tensor_copy(R0[:], PT[:, 0:512])
                            else:
                                nc.vector.tensor_tensor(
                                    R0[:, 512 - w:], R0[:, 512 - w:],
                                    PT[:, off:off + w], add)
                            if idx >= 2:
                                ot_mm(idx - 2)
                        if n >= 2:
                            ot_mm(n - 2)
                        ot_mm(n - 1)
                        # prefetch next tq group's q projection first: the
                        # PE chews on it while the R chain drains, then the
                        # denominator matmul runs without stalling. Its
                        # rope eviction is deferred below the chain.
                        q_next = q_pass_mms(h, g + 1) if g < 3 else None
                        ps1 = pp.tile([1, 512], F32, tag="s2",
                                      name=f"sum_{g}_{h}")
                        nc.tensor.matmul(
                            ps1[:], ones_sb[:], R0[:], start=True, stop=True)
                        s_sb = small_pool.tile([1, 512], F32, tag="s")
                        nc.scalar.copy(s_sb[:], ps1[:])
                        bc = small_pool.tile([128, 512], F32, tag="bc")
                        nc.gpsimd.partition_broadcast(bc[:], s_sb[:])
                        rcp = small_pool.tile([128, 512], F32, tag="rcp")
                        nc.vector.reciprocal_approx_fast(rcp[:], bc[:])
                        nc.vector.tensor_tensor(
                            OTg[:, h, :], pso[:], rcp[:], mult)
                        if q_next is not None:
                            ps_q, q_tsl = q_next
                            rope_evict_dve(ps_q, qT[:, h, q_tsl], q_tsl)
                    # o-proj for this tq group; defer h3 of the first two
                    # units so h3's normalization chain can land
                    units = [(tb, nci) for tb in range(4) for nci in range(4)]
                    psys = {}

                    def oproj_mms(u, hs):
                        tb, nci = u
                        if u not in psys:
                            psys[u] = pp.tile(
                                [128, 512], F32,
                                tag=f"s{units.index(u) % 2}",
                                name=f"y_{g}_{tb}_{nci}")
                        for h in hs:
                            nc.tensor.matmul(
                                psys[u][:],
                                OTg[:, h, tb * 128:(tb + 1) * 128],
                                wo_sb[:, h, nci * 512:(nci + 1) * 512],
                                start=(h == 0), stop=(h == 3),
                                skip_group_check=True)

                    def oproj_evict(u):
                        tb, nci = u
                        ysb = yev_pool.tile([128, 512], FP16, tag="ysb")
                        nonlocal cp
                        if cp % 4 == 0:
                            nc.scalar.copy(ysb[:], psys[u][:])
                        else:
                            nc.vector.tensor_copy(ysb[:], psys[u][:])
                        cp += 1
                        dma_eng = (nc.sync if (cp % 2 == 0 or g == 3)
                                   else nc.gpsimd)
                        dma_eng.dma_start(
                            y_d[512 * g + tb * 128:512 * g + tb * 128 + 128,
                                nci * 512:(nci + 1) * 512],
                            ysb[:])

                    oproj_mms(units[0], [0, 1, 2])
                    oproj_mms(units[1], [0, 1, 2])
                    oproj_mms(units[0], [3])
                    oproj_evict(units[0])
                    oproj_mms(units[1], [3])
                    oproj_evict(units[1])
                    for u in units[2:]:
                        oproj_mms(u, [0, 1, 2, 3])
                        oproj_evict(u)

    nc.compile()
    return nc


def _rope_tables():
    pos = np.arange(T, dtype=np.float32)
    inv_freq = (1.0 / (ROPE_BASE ** (np.arange(0, HD, 2, dtype=np.float32) / HD))).astype(np.float32)
    ang = pos[:, None] * inv_freq[None, :]            # [T, 64]
    cos = np.cos(ang).astype(np.float32)
    sin = np.sin(ang).astype(np.float32)
    cosT = np.ascontiguousarray(np.concatenate([cos, cos], 1).T)   # [128, T]
    sinT = np.ascontiguousarray(np.concatenate([-sin, sin], 1).T)  # rotate_half sign
    return cosT.astype(np.float16), sinT.astype(np.float16)


def kernel(x, Wq, bq, Wk, bk, Wv, bv, Wo, bo, **_ignored):
    x = np.asarray(x, dtype=np.float32)
    Wq = np.asarray(Wq, dtype=np.float32)
    Wk = np.asarray(Wk, dtype=np.float32)
    Wv = np.asarray(Wv, dtype=np.float32)
    Wo = np.asarray(Wo, dtype=np.float32)
    bo = np.asarray(bo, dtype=np.float32)

    if "nc" not in _CACHE:
        _CACHE["nc"] = _build_nc()
    nc = _CACHE["nc"]

    cosT, sinT = _rope_tables()
    # S^T layout: mask[tk, tq] allows tk <= tq within the diagonal block
    triu = np.triu(np.ones((128, 128), dtype=bool))
    mask = np.where(triu, 0.0, -1e9).astype(np.float32)

    in_maps = []
    for c in range(8):
        b, g = c // G, c % G
        xT = x[b].T.astype(np.float16)                  # [D, T]
        xp = np.ascontiguousarray(
            xT.reshape(KC, 128, T).transpose(1, 0, 2))
        wq = Wq[:, g * AQ:(g + 1) * AQ].astype(np.float16)
        wk = Wk[:, g * HD:(g + 1) * HD].astype(np.float16)
        wv = Wv[:, g * HD:(g + 1) * HD].astype(np.float16)
        wcat = np.concatenate([wq, wk, wv], axis=1)     # [D, 768]
        wp = np.ascontiguousarray(
            wcat.reshape(KC, 128, 768).transpose(1, 0, 2))
        wo = np.ascontiguousarray(
            Wo[g * AQ:(g + 1) * AQ, :].astype(np.float16)
            .reshape(4, 128, D).transpose(1, 0, 2))
        in_maps.append({
            "xp": xp,
            "wp": wp,
            "wo": wo,
            "cosT": cosT,
            "sinT": sinT,
            "mask": mask,
        })

    res = run_bass_kernel_spmd(
        nc, in_maps, list(range(8)),
        trace=bool(os.environ.get("KERNEL_TRACE")),
        tmpdir=os.environ.get("KERNEL_TRACE_DIR") or None,
    )
    _CACHE["last_results"] = res

    out = np.zeros((B, T, D), dtype=np.float32)
    for b in range(B):
        acc = np.zeros((T, D), dtype=np.float32)
        for g in range(G):
            acc += res.results[b * G + g]["y"].astype(np.float32)
        out[b] = acc + bo[None, :]
    return out
